# revision 23
# baseline (speedup 1.0000x reference)
"""AnomalyAttention Trainium2 kernel (8 NeuronCores, data-parallel over batch).

Problem (hardcoded): B=32, L=S=512, H=8, E=64, fp32.
Outputs: V [B,L,H,E], series [B,H,L,S] (softmax of QK^T/8), prior [B,H,L,S]
(per-row Gaussian from sigma).

Per-core layout (B_loc = 4 batches per core):
  - Q/K loaded naturally [128, 4, 512] then PE-transposed once per batch into
    QT/KT tiles [128(he), 512(l|s)] so the contraction dim E sits on partitions.
  - scoresT[s,l] = KT_slice.T @ QT_slice (fp32r, 1 cyc/row), exp on ScalarE.
  - AV matmul uses a ones-augmented V stationary [128, 65]: rows 0..63 of the
    PSUM result are V^T (unnormalized), row 64 is the softmax denominator.
  - 1/denom broadcast to all partitions via a K=1 PE matmul; series/V are then
    normalized on VectorE.
  - prior = exp(dist2 * a + lnc) with per-partition scale/bias APs; dist2 is a
    per-core constant.
Outputs are written as series_t [B,H,S,L], prior [B,H,L,S], vout_t [B,H,E,L];
the host transposes series_t/vout_t (cheap numpy view) to the reference layout.
"""

import numpy as np
from contextlib import ExitStack

from concourse import bass, bacc, tile, mybir
from concourse.masks import make_identity

F32 = mybir.dt.float32
F32R = mybir.dt.float32r
I32 = mybir.dt.int32
AF = mybir.ActivationFunctionType

B, L, H, E = 32, 512, 8, 64
N_CORES = 8
B_LOC = B // N_CORES
P = 128
NLB = L // P  # 4 l-blocks (and s-blocks)

BANDW = 192  # prior band window width; window lo per l-block below
BAND_LOS = [0, 96, 224, 320]  # max(0, min(lb*128 - 32, L - BANDW))
LN3 = 1.0986122886681098
LN_SQRT_2PI = 0.9189385332046727
SCALE = 1.0 / 8.0  # 1/sqrt(E)


def r(ap):
    """View an AP as float32r for full-rate PE matmul."""
    return ap.bitcast(F32R)


def _taylor_fix(nc, sigp, s3b, zt, b):
    """Replace s3b with the cubic-Taylor expm1 where z < 0.1."""
    tay = sigp.tile([P, NLB, H], F32, tag=f"tay{b}")
    nc.vector.tensor_scalar(
        tay[:, :, :], zt[:, :, :], 1.0 / 3.0, 1.0,
        op0=mybir.AluOpType.mult, op1=mybir.AluOpType.add,
    )
    nc.vector.tensor_mul(tay[:, :, :], tay[:, :, :], zt[:, :, :])
    nc.vector.tensor_scalar(
        tay[:, :, :], tay[:, :, :], 0.5, 1.0,
        op0=mybir.AluOpType.mult, op1=mybir.AluOpType.add,
    )
    nc.vector.tensor_mul(tay[:, :, :], tay[:, :, :], zt[:, :, :])
    msk = sigp.tile([P, NLB, H], I32, tag=f"msk{b}")
    nc.vector.tensor_scalar(
        msk[:, :, :], zt[:, :, :], 0.1, None,
        op0=mybir.AluOpType.is_lt,
    )
    nc.vector.copy_predicated(s3b, msk[:, :, :], tay[:, :, :])


def build_kernel(b_loc=B_LOC, n_heads=H, band=None, out_eng=("sync", "sync", "sync"),
                 in_eng="gpsimd", band_pack=False, bufs=None, taylor=True):
    """Build the per-core Bass program.

    band: half-width of the prior diagonal band (None = full prior exp).
    out_eng: DMA-issuing engines for (series, prior, vout) outputs.
    in_eng: DMA engine for input loads.
    band_pack: emit prior as a packed diagonal band [*, L, BANDW] (everything
      outside the window is an exact fp32 zero; host scatters into the zero
      output buffer).
    """
    nc = bacc.Bacc("TRN2", target_bir_lowering=False)
    eng = lambda name: getattr(nc, name)
    ser_eng, pri_eng, vo_eng = (eng(e) for e in out_eng)
    ld_eng = eng(in_eng)
    _bufs = dict(inp=2, qkt=2, expp=4, serp=2, prip=2, voutp=2, sc=2, av=2)
    _bufs.update(bufs or {})
    bufs = _bufs

    q_d = nc.declare_dram_parameter("queries", [b_loc, L, H, E], F32, isOutput=False)
    k_d = nc.declare_dram_parameter("keys", [b_loc, L, H, E], F32, isOutput=False)
    v_d = nc.declare_dram_parameter("values", [b_loc, L, H, E], F32, isOutput=False)
    sg_d = nc.declare_dram_parameter("sigma", [b_loc, L, H], F32, isOutput=False)
    st_d = nc.declare_dram_parameter(
        "series_t", [b_loc, n_heads, L, L], F32, isOutput=True
    )
    if band_pack:
        pr_d = nc.declare_dram_parameter(
            "prior_band", [b_loc, n_heads, L, BANDW], F32, isOutput=True
        )
    else:
        pr_d = nc.declare_dram_parameter(
            "prior", [b_loc, n_heads, L, L], F32, isOutput=True
        )
    vo_d = nc.declare_dram_parameter(
        "vout_t", [b_loc, n_heads, E, L], F32, isOutput=True
    )

    q_ap, k_ap, v_ap, sg_ap = q_d.ap(), k_d.ap(), v_d.ap(), sg_d.ap()
    st_ap, pr_ap, vo_ap = st_d.ap(), pr_d.ap(), vo_d.ap()

    with tile.TileContext(nc) as tc, ExitStack() as ctx:
        consts = ctx.enter_context(tc.tile_pool(name="consts", bufs=1))
        sigp = ctx.enter_context(tc.tile_pool(name="sigp", bufs=1))
        inp = ctx.enter_context(tc.tile_pool(name="inp", bufs=bufs["inp"]))
        qkt = ctx.enter_context(tc.tile_pool(name="qkt", bufs=bufs["qkt"]))
        expp = ctx.enter_context(tc.tile_pool(name="expp", bufs=bufs["expp"]))
        serp = ctx.enter_context(tc.tile_pool(name="serp", bufs=bufs["serp"]))
        prip = ctx.enter_context(tc.tile_pool(name="prip", bufs=bufs["prip"]))
        voutp = ctx.enter_context(tc.tile_pool(name="voutp", bufs=bufs["voutp"]))
        invp = ctx.enter_context(tc.tile_pool(name="invp", bufs=2))
        bcsp = ctx.enter_context(tc.tile_pool(name="bcsp", bufs=2))
        ps_tp = ctx.enter_context(tc.tile_pool(name="ps_tp", bufs=1, space="PSUM"))
        ps_sc = ctx.enter_context(tc.tile_pool(name="ps_sc", bufs=bufs["sc"], space="PSUM"))
        ps_av = ctx.enter_context(tc.tile_pool(name="ps_av", bufs=bufs["av"], space="PSUM"))
        ps_bc = ctx.enter_context(tc.tile_pool(name="ps_bc", bufs=1, space="PSUM"))

        # ---- constants ----
        # dist2[p, lb, j] = (lb*128 + p - j)^2 as f32; single iota (p - j),
        # then DVE shifts/square (gpsimd iota takes few sync waits).
        ii = consts.tile([P, L], I32)
        nc.gpsimd.iota(ii[:, :], pattern=[[-1, L]], base=0, channel_multiplier=1)
        iof = consts.tile([P, L], F32)
        nc.vector.tensor_copy(iof[:, :], ii[:, :])
        dist2 = consts.tile([P, NLB, L], F32)
        for lb in range(NLB):
            nc.vector.tensor_scalar_add(dist2[:, lb, :], iof[:, :], float(lb * P))
        nc.vector.tensor_mul(dist2[:, :, :], dist2[:, :, :], dist2[:, :, :])

        ident = consts.tile([P, P], F32)
        make_identity(nc, ident)

        ones_f = consts.tile([P, P], F32)  # f32 ones source (memset can't write f32r)
        nc.vector.memset(ones_f[:, :], 1.0)
        ones_t = consts.tile([P, P], F32)  # row 64 holds the ones for bcast lhsT
        nc.vector.tensor_copy(r(ones_t[64:65, :]), ones_f[64:65, :])

        # ---- sigma preprocessing for all local batches ----
        # sig[p, lb, b, h];  a = -1/(2*s^2), lnc = -(ln s + ln sqrt(2pi))
        # where s = 3^(sigmoid(5x)+1e-5) - 1
        s3 = sigp.tile([P, NLB, b_loc, H], F32)
        lnc_t = sigp.tile([P, NLB, b_loc, H], F32)
        a_t = sigp.tile([P, NLB, b_loc, H], F32)
        for b in range(b_loc):
            sraw = sigp.tile([P, NLB, H], F32, tag=f"sraw{b}")
            ld_eng.dma_start(
                sraw, sg_ap[b].rearrange("(k p) h -> p k h", p=P)
            )
            s3b = s3[:, :, b, :]
            lncb = lnc_t[:, :, b, :]
            ab = a_t[:, :, b, :]
            nc.scalar.activation(s3b, sraw, AF.Sigmoid, scale=5.0)
            nc.vector.tensor_scalar(
                s3b, s3b, 1e-5, LN3,
                op0=mybir.AluOpType.add, op1=mybir.AluOpType.mult,
            )
            # s = expm1(z): exp path for z >= 0.1; cubic Taylor for small z
            # (avoids the exp(z)-1 cancellation that costs ~1% sigma error
            # at z ~ 1e-5)
            zt = sigp.tile([P, NLB, H], F32, tag=f"zt{b}")
            nc.vector.tensor_copy(zt[:, :, :], s3b)
            nc.scalar.activation(s3b, s3b, AF.Exp)
            nc.vector.tensor_scalar_add(s3b, s3b, -1.0)
            if not taylor:
                pass
            else:
                _taylor_fix(nc, sigp, s3b, zt, b)

            nc.scalar.activation(lncb, s3b, AF.Ln)
            nc.vector.tensor_scalar(
                lncb, lncb, LN_SQRT_2PI, -1.0,
                op0=mybir.AluOpType.add, op1=mybir.AluOpType.mult,
            )
            nc.vector.tensor_mul(ab, s3b, s3b)
            nc.vector.reciprocal(ab, ab)
            nc.vector.tensor_scalar_mul(ab, ab, -0.5)

        pri_bufs = None
        if band is not None:
            pri_bufs = []
            for i in range(2):
                prib = prip.tile([P, NLB, L], F32, tag=f"prib{i}")
                pri_bufs.append(prib)
            for t in pri_bufs:
                nc.vector.memset(t, 0.0)

        # ---- main loop ----
        for b in range(b_loc):
            # load Q/K naturally: [p(l within blk), lblk, he]
            qnat = inp.tile([P, NLB, L], F32, tag="qnat")
            ld_eng.dma_start(
                qnat, q_ap[b].rearrange("(k p) h e -> p k (h e)", p=P)
            )
            knat = inp.tile([P, NLB, L], F32, tag="knat")
            ld_eng.dma_start(
                knat, k_ap[b].rearrange("(k p) h e -> p k (h e)", p=P)
            )
            # V with a ones column: [p(s within blk), sblk, h, 65]
            vnat = inp.tile([P, NLB, L], F32, tag="vnat")
            ld_eng.dma_start(
                vnat, v_ap[b].rearrange("(k p) h e -> p k (h e)", p=P)
            )
            vaug = inp.tile([P, NLB, H, E + 1], F32, tag="vaug")
            nc.vector.tensor_copy(
                r(vaug[:, :, :, 0:E]),
                vnat[:, :, :].rearrange("p k (h e) -> p k h e", e=E),
            )
            nc.vector.tensor_copy(
                r(vaug[:, :, :, E]),
                ones_f[:, 0 : NLB * H].rearrange("p (k h) -> p k h", k=NLB),
            )

            # PE-transpose Q,K:  qt[j][he_p, l] with he rows j*128..j*128+127
            qts, kts = [], []
            for name, nat, lst in (("q", qnat, qts), ("k", knat, kts)):
                for j in range(NLB):
                    pt = ps_tp.tile([P, L], F32, tag="tp")
                    for i in range(NLB):
                        nc.tensor.transpose(
                            pt[:, i * P : (i + 1) * P],
                            nat[:, i, j * P : (j + 1) * P],
                            ident[:, :],
                        )
                    t = qkt.tile([P, L], F32, tag=f"{name}t{j}")
                    nc.any.tensor_copy(r(t[:, :]), pt[:, :])
                    lst.append(t)

            vout = voutp.tile([E, n_heads, L], F32)

            for h in range(n_heads):
                g, po = h // 2, (h % 2) * 64
                qt_s = qts[g][po : po + 64, :]  # [64, 512] = Q^T for head h
                av = ps_av.tile([E + 1, L], F32)
                exps = []
                for sp in range(2):
                    sc = ps_sc.tile([P, 2, L], F32)
                    for si in range(2):
                        sblk = 2 * sp + si
                        kt_s = kts[g][po : po + 64, sblk * P : (sblk + 1) * P]
                        nc.tensor.matmul(
                            sc[:, si, :], r(kt_s), r(qt_s), start=True, stop=True
                        )
                    ex = expp.tile([P, 2, L], F32, tag="expt")
                    nc.scalar.activation(r(ex[:, :, :]), sc, AF.Exp, scale=SCALE)
                    exps.append(ex)
                    for si in range(2):
                        sblk = 2 * sp + si
                        nc.tensor.matmul(
                            av[:, :],
                            r(vaug[:, sblk, h, :]),
                            r(ex[:, si, :]),
                            start=(sblk == 0),
                            stop=(sblk == NLB - 1),
                        )
                # denominator -> reciprocal -> broadcast to 128 partitions
                inv = invp.tile([65, L], F32)
                with nc.allow_low_precision(reason="fp32r rounding for PE bcast"):
                    nc.vector.reciprocal(r(inv[64:65, :]), av[64:65, :])
                bc = ps_bc.tile([P, L], F32)
                nc.tensor.matmul(
                    bc[:, :], r(ones_t[64:65, :]), r(inv[64:65, :]),
                    start=True, stop=True,
                )
                bcs = bcsp.tile([P, L], F32)
                nc.any.tensor_copy(bcs[:, :], bc[:, :])
                # series (transposed layout [s, l]) + V
                ser = serp.tile([P, NLB, L], F32)
                for sp in range(2):
                    for si in range(2):
                        sblk = 2 * sp + si
                        nc.vector.tensor_mul(
                            ser[:, sblk, :], exps[sp][:, si, :], bcs[:, :]
                        )
                ser_eng.dma_start(
                    st_ap[b, h].rearrange("(k p) l -> p k l", p=P), ser
                )
                nc.vector.tensor_mul(vout[:, h, :], av[0:E, :], bcs[0:E, :])

                # prior
                if band_pack:
                    pri = prip.tile([P, NLB, BANDW], F32)
                    for lb in range(NLB):
                        lo = BAND_LOS[lb]
                        nc.scalar.activation(
                            pri[:, lb, :],
                            dist2[:, lb, lo : lo + BANDW],
                            AF.Exp,
                            scale=a_t[:, lb, b, h : h + 1],
                            bias=lnc_t[:, lb, b, h : h + 1],
                        )
                    pri_eng.dma_start(
                        pr_ap[b, h].rearrange("(k p) w -> p k w", p=P), pri
                    )
                else:
                    if band is None:
                        pri = prip.tile([P, NLB, L], F32)
                    else:
                        pri = pri_bufs[(b * n_heads + h) % 2]
                    for lb in range(NLB):
                        if band is None:
                            lo, hi = 0, L
                        else:
                            lo = max(0, lb * P - band)
                            hi = min(L, (lb + 1) * P + band)
                        nc.scalar.activation(
                            pri[:, lb, lo:hi],
                            dist2[:, lb, lo:hi],
                            AF.Exp,
                            scale=a_t[:, lb, b, h : h + 1],
                            bias=lnc_t[:, lb, b, h : h + 1],
                        )
                    pri_eng.dma_start(
                        pr_ap[b, h].rearrange("(k p) s -> p k s", p=P), pri
                    )

            vo_eng.dma_start(vo_ap[b].rearrange("h d l -> d h l"), vout)

    nc.compile()
    return nc


TRACE = False  # set True (e.g. from test.py) to capture an NTFF profile
LAST_RESULTS = None  # full BassKernelResults of the most recent run
CONFIG = dict(band_pack=True, bufs=dict(serp=4, prip=4, voutp=1), taylor=False)


def kernel(queries, keys, values, sigma):
    global LAST_RESULTS
    from concourse.bass_utils import run_bass_kernel_spmd

    queries = np.ascontiguousarray(queries, dtype=np.float32)
    keys = np.ascontiguousarray(keys, dtype=np.float32)
    values = np.ascontiguousarray(values, dtype=np.float32)
    sigma = np.ascontiguousarray(sigma, dtype=np.float32)

    nc = build_kernel(**CONFIG)
    in_maps = []
    for i in range(N_CORES):
        s = slice(i * B_LOC, (i + 1) * B_LOC)
        in_maps.append(
            {
                "queries": queries[s],
                "keys": keys[s],
                "values": values[s],
                "sigma": sigma[s],
            }
        )
    res = run_bass_kernel_spmd(
        nc, in_maps, core_ids=list(range(N_CORES)), trace=TRACE
    )
    LAST_RESULTS = res
    results = res.results

    series_t = np.concatenate([r_["series_t"] for r_ in results], axis=0)
    vout_t = np.concatenate([r_["vout_t"] for r_ in results], axis=0)
    if CONFIG.get("band_pack"):
        pb = np.concatenate([r_["prior_band"] for r_ in results], axis=0)
        prior = np.zeros((B, H, L, L), dtype=np.float32)
        for lb, lo in enumerate(BAND_LOS):
            prior[:, :, lb * P : (lb + 1) * P, lo : lo + BANDW] = pb[
                :, :, lb * P : (lb + 1) * P, :
            ]
    else:
        prior = np.concatenate([r_["prior"] for r_ in results], axis=0)

    V = vout_t.transpose(0, 3, 1, 2)  # [B,H,E,L] -> [B,L,H,E]
    series = series_t.transpose(0, 1, 3, 2)  # [B,H,S,L] -> [B,H,L,S]
    return V, series, prior


# revision 24
# speedup vs baseline: 1.0211x; 1.0211x over previous
"""AnomalyAttention Trainium2 kernel (8 NeuronCores, data-parallel over batch).

Problem (hardcoded): B=32, L=S=512, H=8, E=64, fp32.
Outputs: V [B,L,H,E], series [B,H,L,S] (softmax of QK^T/8), prior [B,H,L,S]
(per-row Gaussian from sigma).

Per-core layout (B_loc = 4 batches per core):
  - Q/K loaded naturally [128, 4, 512] then PE-transposed once per batch into
    QT/KT tiles [128(he), 512(l|s)] so the contraction dim E sits on partitions.
  - scoresT[s,l] = KT_slice.T @ QT_slice (fp32r, 1 cyc/row), exp on ScalarE.
  - AV matmul uses a ones-augmented V stationary [128, 65]: rows 0..63 of the
    PSUM result are V^T (unnormalized), row 64 is the softmax denominator.
  - 1/denom broadcast to all partitions via a K=1 PE matmul; series/V are then
    normalized on VectorE.
  - prior = exp(dist2 * a + lnc) with per-partition scale/bias APs; dist2 is a
    per-core constant.
Outputs are written as series_t [B,H,S,L], prior [B,H,L,S], vout_t [B,H,E,L];
the host transposes series_t/vout_t (cheap numpy view) to the reference layout.
"""

import numpy as np
from contextlib import ExitStack

from concourse import bass, bacc, tile, mybir
from concourse.masks import make_identity

F32 = mybir.dt.float32
F32R = mybir.dt.float32r
I32 = mybir.dt.int32
AF = mybir.ActivationFunctionType

B, L, H, E = 32, 512, 8, 64
N_CORES = 8
B_LOC = B // N_CORES
P = 128
NLB = L // P  # 4 l-blocks (and s-blocks)

BANDW = 192  # prior band window width; window lo per l-block below
BAND_LOS = [0, 96, 224, 320]  # max(0, min(lb*128 - 32, L - BANDW))
LN3 = 1.0986122886681098
LN_SQRT_2PI = 0.9189385332046727
SCALE = 1.0 / 8.0  # 1/sqrt(E)


def r(ap):
    """View an AP as float32r for full-rate PE matmul."""
    return ap.bitcast(F32R)


def _taylor_fix(nc, sigp, s3b, zt, b):
    """Replace s3b with the cubic-Taylor expm1 where z < 0.1."""
    tay = sigp.tile([P, NLB, H], F32, tag=f"tay{b}")
    nc.vector.tensor_scalar(
        tay[:, :, :], zt[:, :, :], 1.0 / 3.0, 1.0,
        op0=mybir.AluOpType.mult, op1=mybir.AluOpType.add,
    )
    nc.vector.tensor_mul(tay[:, :, :], tay[:, :, :], zt[:, :, :])
    nc.vector.tensor_scalar(
        tay[:, :, :], tay[:, :, :], 0.5, 1.0,
        op0=mybir.AluOpType.mult, op1=mybir.AluOpType.add,
    )
    nc.vector.tensor_mul(tay[:, :, :], tay[:, :, :], zt[:, :, :])
    msk = sigp.tile([P, NLB, H], I32, tag=f"msk{b}")
    nc.vector.tensor_scalar(
        msk[:, :, :], zt[:, :, :], 0.1, None,
        op0=mybir.AluOpType.is_lt,
    )
    nc.vector.copy_predicated(s3b, msk[:, :, :], tay[:, :, :])


def build_kernel(b_loc=B_LOC, n_heads=H, band=None, out_eng=("sync", "sync", "sync"),
                 in_eng="gpsimd", band_pack=False, bufs=None, taylor=True):
    """Build the per-core Bass program.

    band: half-width of the prior diagonal band (None = full prior exp).
    out_eng: DMA-issuing engines for (series, prior, vout) outputs.
    in_eng: DMA engine for input loads.
    band_pack: emit prior as a packed diagonal band [*, L, BANDW] (everything
      outside the window is an exact fp32 zero; host scatters into the zero
      output buffer).
    """
    nc = bacc.Bacc("TRN2", target_bir_lowering=False)
    eng = lambda name: getattr(nc, name)
    ser_eng, pri_eng, vo_eng = (eng(e) for e in out_eng)
    ld_eng = eng(in_eng)
    _bufs = dict(inp=2, qkt=2, expp=4, serp=2, prip=2, voutp=2, sc=2, av=2)
    _bufs.update(bufs or {})
    bufs = _bufs

    q_d = nc.declare_dram_parameter("queries", [b_loc, L, H, E], F32, isOutput=False)
    k_d = nc.declare_dram_parameter("keys", [b_loc, L, H, E], F32, isOutput=False)
    v_d = nc.declare_dram_parameter("values", [b_loc, L, H, E], F32, isOutput=False)
    sg_d = nc.declare_dram_parameter("sigma", [b_loc, L, H], F32, isOutput=False)
    st_d = nc.declare_dram_parameter(
        "series_t", [b_loc, n_heads, L, L], F32, isOutput=True
    )
    if band_pack:
        pr_d = nc.declare_dram_parameter(
            "prior_band", [b_loc, n_heads, L, BANDW], F32, isOutput=True
        )
    else:
        pr_d = nc.declare_dram_parameter(
            "prior", [b_loc, n_heads, L, L], F32, isOutput=True
        )
    vo_d = nc.declare_dram_parameter(
        "vout_t", [b_loc, n_heads, E, L], F32, isOutput=True
    )

    q_ap, k_ap, v_ap, sg_ap = q_d.ap(), k_d.ap(), v_d.ap(), sg_d.ap()
    st_ap, pr_ap, vo_ap = st_d.ap(), pr_d.ap(), vo_d.ap()

    with tile.TileContext(nc) as tc, ExitStack() as ctx:
        consts = ctx.enter_context(tc.tile_pool(name="consts", bufs=1))
        sigp = ctx.enter_context(tc.tile_pool(name="sigp", bufs=1))
        inp = ctx.enter_context(tc.tile_pool(name="inp", bufs=bufs["inp"]))
        qkt = ctx.enter_context(tc.tile_pool(name="qkt", bufs=bufs["qkt"]))
        expp = ctx.enter_context(tc.tile_pool(name="expp", bufs=bufs["expp"]))
        serp = ctx.enter_context(tc.tile_pool(name="serp", bufs=bufs["serp"]))
        prip = ctx.enter_context(tc.tile_pool(name="prip", bufs=bufs["prip"]))
        voutp = ctx.enter_context(tc.tile_pool(name="voutp", bufs=bufs["voutp"]))
        invp = ctx.enter_context(tc.tile_pool(name="invp", bufs=2))
        bcsp = ctx.enter_context(tc.tile_pool(name="bcsp", bufs=2))
        ps_tp = ctx.enter_context(tc.tile_pool(name="ps_tp", bufs=1, space="PSUM"))
        ps_sc = ctx.enter_context(tc.tile_pool(name="ps_sc", bufs=bufs["sc"], space="PSUM"))
        ps_av = ctx.enter_context(tc.tile_pool(name="ps_av", bufs=bufs["av"], space="PSUM"))
        ps_bc = ctx.enter_context(tc.tile_pool(name="ps_bc", bufs=1, space="PSUM"))

        # ---- constants ----
        # dist2[p, lb, j] = (lb*128 + p - j)^2 as f32; single iota (p - j),
        # then DVE shifts/square (gpsimd iota takes few sync waits).
        ii = consts.tile([P, L], I32)
        nc.gpsimd.iota(ii[:, :], pattern=[[-1, L]], base=0, channel_multiplier=1)
        iof = consts.tile([P, L], F32)
        nc.vector.tensor_copy(iof[:, :], ii[:, :])
        dist2 = consts.tile([P, NLB, L], F32)
        for lb in range(NLB):
            nc.vector.tensor_scalar_add(dist2[:, lb, :], iof[:, :], float(lb * P))
        nc.vector.tensor_mul(dist2[:, :, :], dist2[:, :, :], dist2[:, :, :])

        ident = consts.tile([P, P], F32)
        make_identity(nc, ident)

        ones_f = consts.tile([P, P], F32)  # f32 ones source (memset can't write f32r)
        nc.vector.memset(ones_f[:, :], 1.0)
        ones_t = consts.tile([P, P], F32)  # row 64 holds the ones for bcast lhsT
        nc.vector.tensor_copy(r(ones_t[64:65, :]), ones_f[64:65, :])

        # ---- sigma preprocessing for all local batches ----
        # sig[p, lb, b, h];  a = -1/(2*s^2), lnc = -(ln s + ln sqrt(2pi))
        # where s = 3^(sigmoid(5x)+1e-5) - 1
        s3 = sigp.tile([P, NLB, b_loc, H], F32)
        lnc_t = sigp.tile([P, NLB, b_loc, H], F32)
        a_t = sigp.tile([P, NLB, b_loc, H], F32)
        for b in range(b_loc):
            sraw = sigp.tile([P, NLB, H], F32, tag=f"sraw{b}")
            ld_eng.dma_start(
                sraw, sg_ap[b].rearrange("(k p) h -> p k h", p=P)
            )
            s3b = s3[:, :, b, :]
            lncb = lnc_t[:, :, b, :]
            ab = a_t[:, :, b, :]
            nc.scalar.activation(s3b, sraw, AF.Sigmoid, scale=5.0)
            nc.vector.tensor_scalar(
                s3b, s3b, 1e-5, LN3,
                op0=mybir.AluOpType.add, op1=mybir.AluOpType.mult,
            )
            # s = expm1(z): exp path for z >= 0.1; cubic Taylor for small z
            # (avoids the exp(z)-1 cancellation that costs ~1% sigma error
            # at z ~ 1e-5)
            if taylor:
                zt = sigp.tile([P, NLB, H], F32, tag=f"zt{b}")
                nc.vector.tensor_copy(zt[:, :, :], s3b)
            nc.scalar.activation(s3b, s3b, AF.Exp)
            nc.vector.tensor_scalar_add(s3b, s3b, -1.0)
            if taylor:
                _taylor_fix(nc, sigp, s3b, zt, b)

            nc.scalar.activation(lncb, s3b, AF.Ln)
            nc.vector.tensor_scalar(
                lncb, lncb, LN_SQRT_2PI, -1.0,
                op0=mybir.AluOpType.add, op1=mybir.AluOpType.mult,
            )
            nc.vector.tensor_mul(ab, s3b, s3b)
            nc.vector.reciprocal(ab, ab)
            nc.vector.tensor_scalar_mul(ab, ab, -0.5)

        pri_bufs = None
        if band is not None:
            pri_bufs = []
            for i in range(2):
                prib = prip.tile([P, NLB, L], F32, tag=f"prib{i}")
                pri_bufs.append(prib)
            for t in pri_bufs:
                nc.vector.memset(t, 0.0)

        # ---- main loop ----
        for b in range(b_loc):
            # load Q/K naturally: [p(l within blk), lblk, he]
            qnat = inp.tile([P, NLB, L], F32, tag="qnat")
            ld_eng.dma_start(
                qnat, q_ap[b].rearrange("(k p) h e -> p k (h e)", p=P)
            )
            knat = inp.tile([P, NLB, L], F32, tag="knat")
            ld_eng.dma_start(
                knat, k_ap[b].rearrange("(k p) h e -> p k (h e)", p=P)
            )
            # V with a ones column: [p(s within blk), sblk, h, 65]
            vnat = inp.tile([P, NLB, L], F32, tag="vnat")
            ld_eng.dma_start(
                vnat, v_ap[b].rearrange("(k p) h e -> p k (h e)", p=P)
            )
            vaug = inp.tile([P, NLB, H, E + 1], F32, tag="vaug")
            nc.vector.tensor_copy(
                r(vaug[:, :, :, 0:E]),
                vnat[:, :, :].rearrange("p k (h e) -> p k h e", e=E),
            )
            nc.vector.tensor_copy(
                r(vaug[:, :, :, E]),
                ones_f[:, 0 : NLB * H].rearrange("p (k h) -> p k h", k=NLB),
            )

            # PE-transpose Q,K:  qt[j][he_p, l] with he rows j*128..j*128+127
            qts, kts = [], []
            for name, nat, lst in (("q", qnat, qts), ("k", knat, kts)):
                for j in range(NLB):
                    pt = ps_tp.tile([P, L], F32, tag="tp")
                    for i in range(NLB):
                        nc.tensor.transpose(
                            pt[:, i * P : (i + 1) * P],
                            nat[:, i, j * P : (j + 1) * P],
                            ident[:, :],
                        )
                    t = qkt.tile([P, L], F32, tag=f"{name}t{j}")
                    nc.any.tensor_copy(r(t[:, :]), pt[:, :])
                    lst.append(t)

            vout = voutp.tile([E, n_heads, L], F32)

            for h in range(n_heads):
                g, po = h // 2, (h % 2) * 64
                qt_s = qts[g][po : po + 64, :]  # [64, 512] = Q^T for head h
                av = ps_av.tile([E + 1, L], F32)
                exps = []
                for sp in range(2):
                    sc = ps_sc.tile([P, 2, L], F32)
                    for si in range(2):
                        sblk = 2 * sp + si
                        kt_s = kts[g][po : po + 64, sblk * P : (sblk + 1) * P]
                        nc.tensor.matmul(
                            sc[:, si, :], r(kt_s), r(qt_s), start=True, stop=True
                        )
                    ex = expp.tile([P, 2, L], F32, tag="expt")
                    nc.scalar.activation(r(ex[:, :, :]), sc, AF.Exp, scale=SCALE)
                    exps.append(ex)
                    for si in range(2):
                        sblk = 2 * sp + si
                        nc.tensor.matmul(
                            av[:, :],
                            r(vaug[:, sblk, h, :]),
                            r(ex[:, si, :]),
                            start=(sblk == 0),
                            stop=(sblk == NLB - 1),
                        )
                # denominator -> reciprocal -> broadcast to 128 partitions
                inv = invp.tile([65, L], F32)
                with nc.allow_low_precision(reason="fp32r rounding for PE bcast"):
                    nc.vector.reciprocal(r(inv[64:65, :]), av[64:65, :])
                bc = ps_bc.tile([P, L], F32)
                nc.tensor.matmul(
                    bc[:, :], r(ones_t[64:65, :]), r(inv[64:65, :]),
                    start=True, stop=True,
                )
                bcs = bcsp.tile([P, L], F32)
                nc.any.tensor_copy(bcs[:, :], bc[:, :])
                # series (transposed layout [s, l]) + V
                ser = serp.tile([P, NLB, L], F32)
                for sp in range(2):
                    for si in range(2):
                        sblk = 2 * sp + si
                        nc.vector.tensor_mul(
                            ser[:, sblk, :], exps[sp][:, si, :], bcs[:, :]
                        )
                ser_eng.dma_start(
                    st_ap[b, h].rearrange("(k p) l -> p k l", p=P), ser
                )
                nc.vector.tensor_mul(vout[:, h, :], av[0:E, :], bcs[0:E, :])

                # prior
                if band_pack:
                    pri = prip.tile([P, NLB, BANDW], F32)
                    for lb in range(NLB):
                        lo = BAND_LOS[lb]
                        nc.scalar.activation(
                            pri[:, lb, :],
                            dist2[:, lb, lo : lo + BANDW],
                            AF.Exp,
                            scale=a_t[:, lb, b, h : h + 1],
                            bias=lnc_t[:, lb, b, h : h + 1],
                        )
                    pri_eng.dma_start(
                        pr_ap[b, h].rearrange("(k p) w -> p k w", p=P), pri
                    )
                else:
                    if band is None:
                        pri = prip.tile([P, NLB, L], F32)
                    else:
                        pri = pri_bufs[(b * n_heads + h) % 2]
                    for lb in range(NLB):
                        if band is None:
                            lo, hi = 0, L
                        else:
                            lo = max(0, lb * P - band)
                            hi = min(L, (lb + 1) * P + band)
                        nc.scalar.activation(
                            pri[:, lb, lo:hi],
                            dist2[:, lb, lo:hi],
                            AF.Exp,
                            scale=a_t[:, lb, b, h : h + 1],
                            bias=lnc_t[:, lb, b, h : h + 1],
                        )
                    pri_eng.dma_start(
                        pr_ap[b, h].rearrange("(k p) s -> p k s", p=P), pri
                    )

            vo_eng.dma_start(vo_ap[b].rearrange("h d l -> d h l"), vout)

    nc.compile()
    return nc


TRACE = False  # set True (e.g. from test.py) to capture an NTFF profile
LAST_RESULTS = None  # full BassKernelResults of the most recent run
CONFIG = dict(band_pack=True, bufs=dict(serp=4, prip=4, voutp=1), taylor=False)


def kernel(queries, keys, values, sigma):
    global LAST_RESULTS
    from concourse.bass_utils import run_bass_kernel_spmd

    queries = np.ascontiguousarray(queries, dtype=np.float32)
    keys = np.ascontiguousarray(keys, dtype=np.float32)
    values = np.ascontiguousarray(values, dtype=np.float32)
    sigma = np.ascontiguousarray(sigma, dtype=np.float32)

    nc = build_kernel(**CONFIG)
    in_maps = []
    for i in range(N_CORES):
        s = slice(i * B_LOC, (i + 1) * B_LOC)
        in_maps.append(
            {
                "queries": queries[s],
                "keys": keys[s],
                "values": values[s],
                "sigma": sigma[s],
            }
        )
    res = run_bass_kernel_spmd(
        nc, in_maps, core_ids=list(range(N_CORES)), trace=TRACE
    )
    LAST_RESULTS = res
    results = res.results

    series_t = np.concatenate([r_["series_t"] for r_ in results], axis=0)
    vout_t = np.concatenate([r_["vout_t"] for r_ in results], axis=0)
    if CONFIG.get("band_pack"):
        pb = np.concatenate([r_["prior_band"] for r_ in results], axis=0)
        prior = np.zeros((B, H, L, L), dtype=np.float32)
        for lb, lo in enumerate(BAND_LOS):
            prior[:, :, lb * P : (lb + 1) * P, lo : lo + BANDW] = pb[
                :, :, lb * P : (lb + 1) * P, :
            ]
    else:
        prior = np.concatenate([r_["prior"] for r_ in results], axis=0)

    V = vout_t.transpose(0, 3, 1, 2)  # [B,H,E,L] -> [B,L,H,E]
    series = series_t.transpose(0, 1, 3, 2)  # [B,H,S,L] -> [B,H,L,S]
    return V, series, prior


# revision 25
# speedup vs baseline: 1.0275x; 1.0063x over previous
"""AnomalyAttention Trainium2 kernel (8 NeuronCores, data-parallel over batch).

Problem (hardcoded): B=32, L=S=512, H=8, E=64, fp32.
Outputs: V [B,L,H,E], series [B,H,L,S] (softmax of QK^T/8), prior [B,H,L,S]
(per-row Gaussian from sigma).

Per-core layout (B_loc = 4 batches per core):
  - Q/K loaded naturally [128, 4, 512] then PE-transposed once per batch into
    QT/KT tiles [128(he), 512(l|s)] so the contraction dim E sits on partitions.
  - scoresT[s,l] = KT_slice.T @ QT_slice (fp32r, 1 cyc/row), exp on ScalarE.
  - AV matmul uses a ones-augmented V stationary [128, 65]: rows 0..63 of the
    PSUM result are V^T (unnormalized), row 64 is the softmax denominator.
  - 1/denom broadcast to all partitions via a K=1 PE matmul; series/V are then
    normalized on VectorE.
  - prior = exp(dist2 * a + lnc) with per-partition scale/bias APs; dist2 is a
    per-core constant.
Outputs are written as series_t [B,H,S,L], prior [B,H,L,S], vout_t [B,H,E,L];
the host transposes series_t/vout_t (cheap numpy view) to the reference layout.
"""

import numpy as np
from contextlib import ExitStack

from concourse import bass, bacc, tile, mybir
from concourse.masks import make_identity

F32 = mybir.dt.float32
F32R = mybir.dt.float32r
I32 = mybir.dt.int32
AF = mybir.ActivationFunctionType

B, L, H, E = 32, 512, 8, 64
N_CORES = 8
B_LOC = B // N_CORES
P = 128
NLB = L // P  # 4 l-blocks (and s-blocks)

# Prior band window: sigma <= 2.0000 so at |l-s| >= 17 the prior is at most
# 0.2*exp(-17^2/8) ~ 5e-17 (12 orders below the output scale); cells outside a
# +-16 band are left as zeros from the pre-zeroed output buffer.
BANDW = 160  # 128 + 2*16
BAND_LOS = [0, 112, 240, 352]  # max(0, min(lb*128 - 16, L - BANDW))
LN3 = 1.0986122886681098
LN_SQRT_2PI = 0.9189385332046727
SCALE = 1.0 / 8.0  # 1/sqrt(E)


def r(ap):
    """View an AP as float32r for full-rate PE matmul."""
    return ap.bitcast(F32R)


def _taylor_fix(nc, sigp, s3b, zt, b):
    """Replace s3b with the cubic-Taylor expm1 where z < 0.1."""
    tay = sigp.tile([P, NLB, H], F32, tag=f"tay{b}")
    nc.vector.tensor_scalar(
        tay[:, :, :], zt[:, :, :], 1.0 / 3.0, 1.0,
        op0=mybir.AluOpType.mult, op1=mybir.AluOpType.add,
    )
    nc.vector.tensor_mul(tay[:, :, :], tay[:, :, :], zt[:, :, :])
    nc.vector.tensor_scalar(
        tay[:, :, :], tay[:, :, :], 0.5, 1.0,
        op0=mybir.AluOpType.mult, op1=mybir.AluOpType.add,
    )
    nc.vector.tensor_mul(tay[:, :, :], tay[:, :, :], zt[:, :, :])
    msk = sigp.tile([P, NLB, H], I32, tag=f"msk{b}")
    nc.vector.tensor_scalar(
        msk[:, :, :], zt[:, :, :], 0.1, None,
        op0=mybir.AluOpType.is_lt,
    )
    nc.vector.copy_predicated(s3b, msk[:, :, :], tay[:, :, :])


def build_kernel(b_loc=B_LOC, n_heads=H, band=None, out_eng=("sync", "sync", "sync"),
                 in_eng="gpsimd", band_pack=False, bufs=None, taylor=True):
    """Build the per-core Bass program.

    band: half-width of the prior diagonal band (None = full prior exp).
    out_eng: DMA-issuing engines for (series, prior, vout) outputs.
    in_eng: DMA engine for input loads.
    band_pack: emit prior as a packed diagonal band [*, L, BANDW] (everything
      outside the window is an exact fp32 zero; host scatters into the zero
      output buffer).
    """
    nc = bacc.Bacc("TRN2", target_bir_lowering=False)
    eng = lambda name: getattr(nc, name)
    ser_eng, pri_eng, vo_eng = (eng(e) for e in out_eng)
    ld_eng = eng(in_eng)
    _bufs = dict(inp=2, qkt=2, expp=4, serp=2, prip=2, voutp=2, sc=2, av=2)
    _bufs.update(bufs or {})
    bufs = _bufs

    q_d = nc.declare_dram_parameter("queries", [b_loc, L, H, E], F32, isOutput=False)
    k_d = nc.declare_dram_parameter("keys", [b_loc, L, H, E], F32, isOutput=False)
    v_d = nc.declare_dram_parameter("values", [b_loc, L, H, E], F32, isOutput=False)
    sg_d = nc.declare_dram_parameter("sigma", [b_loc, L, H], F32, isOutput=False)
    st_d = nc.declare_dram_parameter(
        "series_t", [b_loc, n_heads, L, L], F32, isOutput=True
    )
    if band_pack:
        pr_d = nc.declare_dram_parameter(
            "prior_band", [b_loc, n_heads, L, BANDW], F32, isOutput=True
        )
    else:
        pr_d = nc.declare_dram_parameter(
            "prior", [b_loc, n_heads, L, L], F32, isOutput=True
        )
    vo_d = nc.declare_dram_parameter(
        "vout_t", [b_loc, n_heads, E, L], F32, isOutput=True
    )

    q_ap, k_ap, v_ap, sg_ap = q_d.ap(), k_d.ap(), v_d.ap(), sg_d.ap()
    st_ap, pr_ap, vo_ap = st_d.ap(), pr_d.ap(), vo_d.ap()

    with tile.TileContext(nc) as tc, ExitStack() as ctx:
        consts = ctx.enter_context(tc.tile_pool(name="consts", bufs=1))
        sigp = ctx.enter_context(tc.tile_pool(name="sigp", bufs=1))
        inp = ctx.enter_context(tc.tile_pool(name="inp", bufs=bufs["inp"]))
        qkt = ctx.enter_context(tc.tile_pool(name="qkt", bufs=bufs["qkt"]))
        expp = ctx.enter_context(tc.tile_pool(name="expp", bufs=bufs["expp"]))
        serp = ctx.enter_context(tc.tile_pool(name="serp", bufs=bufs["serp"]))
        prip = ctx.enter_context(tc.tile_pool(name="prip", bufs=bufs["prip"]))
        voutp = ctx.enter_context(tc.tile_pool(name="voutp", bufs=bufs["voutp"]))
        invp = ctx.enter_context(tc.tile_pool(name="invp", bufs=2))
        bcsp = ctx.enter_context(tc.tile_pool(name="bcsp", bufs=2))
        ps_tp = ctx.enter_context(tc.tile_pool(name="ps_tp", bufs=1, space="PSUM"))
        ps_sc = ctx.enter_context(tc.tile_pool(name="ps_sc", bufs=bufs["sc"], space="PSUM"))
        ps_av = ctx.enter_context(tc.tile_pool(name="ps_av", bufs=bufs["av"], space="PSUM"))
        ps_bc = ctx.enter_context(tc.tile_pool(name="ps_bc", bufs=1, space="PSUM"))

        # ---- constants ----
        # dist2[p, lb, j] = (lb*128 + p - j)^2 as f32; single iota (p - j),
        # then DVE shifts/square (gpsimd iota takes few sync waits).
        ii = consts.tile([P, L], I32)
        nc.gpsimd.iota(ii[:, :], pattern=[[-1, L]], base=0, channel_multiplier=1)
        iof = consts.tile([P, L], F32)
        nc.vector.tensor_copy(iof[:, :], ii[:, :])
        dist2 = consts.tile([P, NLB, L], F32)
        for lb in range(NLB):
            nc.vector.tensor_scalar_add(dist2[:, lb, :], iof[:, :], float(lb * P))
        nc.vector.tensor_mul(dist2[:, :, :], dist2[:, :, :], dist2[:, :, :])

        ident = consts.tile([P, P], F32)
        make_identity(nc, ident)

        ones_f = consts.tile([P, P], F32)  # f32 ones source (memset can't write f32r)
        nc.vector.memset(ones_f[:, :], 1.0)
        ones_t = consts.tile([P, P], F32)  # row 64 holds the ones for bcast lhsT
        nc.vector.tensor_copy(r(ones_t[64:65, :]), ones_f[64:65, :])

        # ---- sigma preprocessing for all local batches ----
        # sig[p, lb, b, h];  a = -1/(2*s^2), lnc = -(ln s + ln sqrt(2pi))
        # where s = 3^(sigmoid(5x)+1e-5) - 1
        s3 = sigp.tile([P, NLB, b_loc, H], F32)
        lnc_t = sigp.tile([P, NLB, b_loc, H], F32)
        a_t = sigp.tile([P, NLB, b_loc, H], F32)
        for b in range(b_loc):
            sraw = sigp.tile([P, NLB, H], F32, tag=f"sraw{b}")
            ld_eng.dma_start(
                sraw, sg_ap[b].rearrange("(k p) h -> p k h", p=P)
            )
            s3b = s3[:, :, b, :]
            lncb = lnc_t[:, :, b, :]
            ab = a_t[:, :, b, :]
            nc.scalar.activation(s3b, sraw, AF.Sigmoid, scale=5.0)
            nc.vector.tensor_scalar(
                s3b, s3b, 1e-5, LN3,
                op0=mybir.AluOpType.add, op1=mybir.AluOpType.mult,
            )
            # s = expm1(z): exp path for z >= 0.1; cubic Taylor for small z
            # (avoids the exp(z)-1 cancellation that costs ~1% sigma error
            # at z ~ 1e-5)
            if taylor:
                zt = sigp.tile([P, NLB, H], F32, tag=f"zt{b}")
                nc.vector.tensor_copy(zt[:, :, :], s3b)
            nc.scalar.activation(s3b, s3b, AF.Exp)
            nc.vector.tensor_scalar_add(s3b, s3b, -1.0)
            if taylor:
                _taylor_fix(nc, sigp, s3b, zt, b)

            nc.scalar.activation(lncb, s3b, AF.Ln)
            nc.vector.tensor_scalar(
                lncb, lncb, LN_SQRT_2PI, -1.0,
                op0=mybir.AluOpType.add, op1=mybir.AluOpType.mult,
            )
            nc.vector.tensor_mul(ab, s3b, s3b)
            nc.vector.reciprocal(ab, ab)
            nc.vector.tensor_scalar_mul(ab, ab, -0.5)

        pri_bufs = None
        if band is not None:
            pri_bufs = []
            for i in range(2):
                prib = prip.tile([P, NLB, L], F32, tag=f"prib{i}")
                pri_bufs.append(prib)
            for t in pri_bufs:
                nc.vector.memset(t, 0.0)

        # ---- main loop ----
        for b in range(b_loc):
            # load Q/K naturally: [p(l within blk), lblk, he]
            qnat = inp.tile([P, NLB, L], F32, tag="qnat")
            ld_eng.dma_start(
                qnat, q_ap[b].rearrange("(k p) h e -> p k (h e)", p=P)
            )
            knat = inp.tile([P, NLB, L], F32, tag="knat")
            ld_eng.dma_start(
                knat, k_ap[b].rearrange("(k p) h e -> p k (h e)", p=P)
            )
            # V with a ones column: [p(s within blk), sblk, h, 65]
            vnat = inp.tile([P, NLB, L], F32, tag="vnat")
            ld_eng.dma_start(
                vnat, v_ap[b].rearrange("(k p) h e -> p k (h e)", p=P)
            )
            vaug = inp.tile([P, NLB, H, E + 1], F32, tag="vaug")
            nc.vector.tensor_copy(
                r(vaug[:, :, :, 0:E]),
                vnat[:, :, :].rearrange("p k (h e) -> p k h e", e=E),
            )
            nc.vector.tensor_copy(
                r(vaug[:, :, :, E]),
                ones_f[:, 0 : NLB * H].rearrange("p (k h) -> p k h", k=NLB),
            )

            # PE-transpose Q,K:  qt[j][he_p, l] with he rows j*128..j*128+127
            qts, kts = [], []
            for name, nat, lst in (("q", qnat, qts), ("k", knat, kts)):
                for j in range(NLB):
                    pt = ps_tp.tile([P, L], F32, tag="tp")
                    for i in range(NLB):
                        nc.tensor.transpose(
                            pt[:, i * P : (i + 1) * P],
                            nat[:, i, j * P : (j + 1) * P],
                            ident[:, :],
                        )
                    t = qkt.tile([P, L], F32, tag=f"{name}t{j}")
                    nc.any.tensor_copy(r(t[:, :]), pt[:, :])
                    lst.append(t)

            vout = voutp.tile([E, n_heads, L], F32)

            for h in range(n_heads):
                g, po = h // 2, (h % 2) * 64
                qt_s = qts[g][po : po + 64, :]  # [64, 512] = Q^T for head h
                av = ps_av.tile([E + 1, L], F32)
                exps = []
                for sp in range(2):
                    sc = ps_sc.tile([P, 2, L], F32)
                    for si in range(2):
                        sblk = 2 * sp + si
                        kt_s = kts[g][po : po + 64, sblk * P : (sblk + 1) * P]
                        nc.tensor.matmul(
                            sc[:, si, :], r(kt_s), r(qt_s), start=True, stop=True
                        )
                    ex = expp.tile([P, 2, L], F32, tag="expt")
                    nc.scalar.activation(r(ex[:, :, :]), sc, AF.Exp, scale=SCALE)
                    exps.append(ex)
                    for si in range(2):
                        sblk = 2 * sp + si
                        nc.tensor.matmul(
                            av[:, :],
                            r(vaug[:, sblk, h, :]),
                            r(ex[:, si, :]),
                            start=(sblk == 0),
                            stop=(sblk == NLB - 1),
                        )
                # denominator -> reciprocal -> broadcast to 128 partitions
                inv = invp.tile([65, L], F32)
                with nc.allow_low_precision(reason="fp32r rounding for PE bcast"):
                    nc.vector.reciprocal(r(inv[64:65, :]), av[64:65, :])
                bc = ps_bc.tile([P, L], F32)
                nc.tensor.matmul(
                    bc[:, :], r(ones_t[64:65, :]), r(inv[64:65, :]),
                    start=True, stop=True,
                )
                bcs = bcsp.tile([P, L], F32)
                nc.any.tensor_copy(bcs[:, :], bc[:, :])
                # series (transposed layout [s, l]) + V
                ser = serp.tile([P, NLB, L], F32)
                for sp in range(2):
                    for si in range(2):
                        sblk = 2 * sp + si
                        nc.vector.tensor_mul(
                            ser[:, sblk, :], exps[sp][:, si, :], bcs[:, :]
                        )
                ser_eng.dma_start(
                    st_ap[b, h].rearrange("(k p) l -> p k l", p=P), ser
                )
                nc.vector.tensor_mul(vout[:, h, :], av[0:E, :], bcs[0:E, :])

                # prior
                if band_pack:
                    pri = prip.tile([P, NLB, BANDW], F32)
                    for lb in range(NLB):
                        lo = BAND_LOS[lb]
                        nc.scalar.activation(
                            pri[:, lb, :],
                            dist2[:, lb, lo : lo + BANDW],
                            AF.Exp,
                            scale=a_t[:, lb, b, h : h + 1],
                            bias=lnc_t[:, lb, b, h : h + 1],
                        )
                    pri_eng.dma_start(
                        pr_ap[b, h].rearrange("(k p) w -> p k w", p=P), pri
                    )
                else:
                    if band is None:
                        pri = prip.tile([P, NLB, L], F32)
                    else:
                        pri = pri_bufs[(b * n_heads + h) % 2]
                    for lb in range(NLB):
                        if band is None:
                            lo, hi = 0, L
                        else:
                            lo = max(0, lb * P - band)
                            hi = min(L, (lb + 1) * P + band)
                        nc.scalar.activation(
                            pri[:, lb, lo:hi],
                            dist2[:, lb, lo:hi],
                            AF.Exp,
                            scale=a_t[:, lb, b, h : h + 1],
                            bias=lnc_t[:, lb, b, h : h + 1],
                        )
                    pri_eng.dma_start(
                        pr_ap[b, h].rearrange("(k p) s -> p k s", p=P), pri
                    )

            vo_eng.dma_start(vo_ap[b].rearrange("h d l -> d h l"), vout)

    nc.compile()
    return nc


TRACE = False  # set True (e.g. from test.py) to capture an NTFF profile
LAST_RESULTS = None  # full BassKernelResults of the most recent run
CONFIG = dict(band_pack=True, bufs=dict(serp=4, prip=4, voutp=1), taylor=False)


def kernel(queries, keys, values, sigma):
    global LAST_RESULTS
    from concourse.bass_utils import run_bass_kernel_spmd

    queries = np.ascontiguousarray(queries, dtype=np.float32)
    keys = np.ascontiguousarray(keys, dtype=np.float32)
    values = np.ascontiguousarray(values, dtype=np.float32)
    sigma = np.ascontiguousarray(sigma, dtype=np.float32)

    nc = build_kernel(**CONFIG)
    in_maps = []
    for i in range(N_CORES):
        s = slice(i * B_LOC, (i + 1) * B_LOC)
        in_maps.append(
            {
                "queries": queries[s],
                "keys": keys[s],
                "values": values[s],
                "sigma": sigma[s],
            }
        )
    res = run_bass_kernel_spmd(
        nc, in_maps, core_ids=list(range(N_CORES)), trace=TRACE
    )
    LAST_RESULTS = res
    results = res.results

    series_t = np.concatenate([r_["series_t"] for r_ in results], axis=0)
    vout_t = np.concatenate([r_["vout_t"] for r_ in results], axis=0)
    if CONFIG.get("band_pack"):
        pb = np.concatenate([r_["prior_band"] for r_ in results], axis=0)
        prior = np.zeros((B, H, L, L), dtype=np.float32)
        for lb, lo in enumerate(BAND_LOS):
            prior[:, :, lb * P : (lb + 1) * P, lo : lo + BANDW] = pb[
                :, :, lb * P : (lb + 1) * P, :
            ]
    else:
        prior = np.concatenate([r_["prior"] for r_ in results], axis=0)

    V = vout_t.transpose(0, 3, 1, 2)  # [B,H,E,L] -> [B,L,H,E]
    series = series_t.transpose(0, 1, 3, 2)  # [B,H,S,L] -> [B,H,L,S]
    return V, series, prior


# revision 26
# speedup vs baseline: 1.0443x; 1.0164x over previous
"""AnomalyAttention Trainium2 kernel (8 NeuronCores, data-parallel over batch).

Problem (hardcoded): B=32, L=S=512, H=8, E=64, fp32.
Outputs: V [B,L,H,E], series [B,H,L,S] (softmax of QK^T/8), prior [B,H,L,S]
(per-row Gaussian from sigma).

Per-core layout (B_loc = 4 batches per core):
  - Q/K loaded naturally [128, 4, 512] then PE-transposed once per batch into
    QT/KT tiles [128(he), 512(l|s)] so the contraction dim E sits on partitions.
  - scoresT[s,l] = KT_slice.T @ QT_slice (fp32r, 1 cyc/row), exp on ScalarE.
  - AV matmul uses a ones-augmented V stationary [128, 65]: rows 0..63 of the
    PSUM result are V^T (unnormalized), row 64 is the softmax denominator.
  - 1/denom broadcast to all partitions via a K=1 PE matmul; series/V are then
    normalized on VectorE.
  - prior = exp(dist2 * a + lnc) with per-partition scale/bias APs; dist2 is a
    per-core constant.
Outputs are written as series_t [B,H,S,L], prior [B,H,L,S], vout_t [B,H,E,L];
the host transposes series_t/vout_t (cheap numpy view) to the reference layout.
"""

import numpy as np
from contextlib import ExitStack

from concourse import bass, bacc, tile, mybir
from concourse.masks import make_identity

F32 = mybir.dt.float32
F32R = mybir.dt.float32r
I32 = mybir.dt.int32
AF = mybir.ActivationFunctionType

B, L, H, E = 32, 512, 8, 64
N_CORES = 8
B_LOC = B // N_CORES
P = 128
NLB = L // P  # 4 l-blocks (and s-blocks)

# Prior band window: sigma <= 2.0000 so at |l-s| >= 17 the prior is at most
# 0.2*exp(-17^2/8) ~ 5e-17 (12 orders below the output scale); cells outside a
# +-16 band are left as zeros from the pre-zeroed output buffer.
BANDW = 160  # 128 + 2*16
BAND_LOS = [0, 112, 240, 352]  # max(0, min(lb*128 - 16, L - BANDW))
LN3 = 1.0986122886681098
LN_SQRT_2PI = 0.9189385332046727
SCALE = 1.0 / 8.0  # 1/sqrt(E)


def r(ap):
    """View an AP as float32r for full-rate PE matmul."""
    return ap.bitcast(F32R)


def _taylor_fix(nc, sigp, s3b, zt, b):
    """Replace s3b with the cubic-Taylor expm1 where z < 0.1."""
    tay = sigp.tile([P, NLB, H], F32, tag=f"tay{b}")
    nc.vector.tensor_scalar(
        tay[:, :, :], zt[:, :, :], 1.0 / 3.0, 1.0,
        op0=mybir.AluOpType.mult, op1=mybir.AluOpType.add,
    )
    nc.vector.tensor_mul(tay[:, :, :], tay[:, :, :], zt[:, :, :])
    nc.vector.tensor_scalar(
        tay[:, :, :], tay[:, :, :], 0.5, 1.0,
        op0=mybir.AluOpType.mult, op1=mybir.AluOpType.add,
    )
    nc.vector.tensor_mul(tay[:, :, :], tay[:, :, :], zt[:, :, :])
    msk = sigp.tile([P, NLB, H], I32, tag=f"msk{b}")
    nc.vector.tensor_scalar(
        msk[:, :, :], zt[:, :, :], 0.1, None,
        op0=mybir.AluOpType.is_lt,
    )
    nc.vector.copy_predicated(s3b, msk[:, :, :], tay[:, :, :])


def build_kernel(b_loc=B_LOC, n_heads=H, band=None, out_eng=("sync", "sync", "sync"),
                 in_eng="gpsimd", band_pack=False, bufs=None, taylor=True):
    """Build the per-core Bass program.

    band: half-width of the prior diagonal band (None = full prior exp).
    out_eng: DMA-issuing engines for (series, prior, vout) outputs.
    in_eng: DMA engine for input loads.
    band_pack: emit prior as a packed diagonal band [*, L, BANDW] (everything
      outside the window is an exact fp32 zero; host scatters into the zero
      output buffer).
    """
    nc = bacc.Bacc("TRN2", target_bir_lowering=False)
    eng = lambda name: getattr(nc, name)
    ser_eng, pri_eng, vo_eng = (eng(e) for e in out_eng)
    ld_eng = eng(in_eng)
    _bufs = dict(inp=2, qkt=2, expp=4, serp=2, prip=2, voutp=2, sc=2, av=2)
    _bufs.update(bufs or {})
    bufs = _bufs

    q_d = nc.declare_dram_parameter("queries", [b_loc, L, H, E], F32, isOutput=False)
    k_d = nc.declare_dram_parameter("keys", [b_loc, L, H, E], F32, isOutput=False)
    v_d = nc.declare_dram_parameter("values", [b_loc, L, H, E], F32, isOutput=False)
    sg_d = nc.declare_dram_parameter("sigma", [b_loc, L, H], F32, isOutput=False)
    st_d = nc.declare_dram_parameter(
        "series_t", [b_loc, n_heads, L, L], F32, isOutput=True
    )
    if band_pack:
        pr_d = nc.declare_dram_parameter(
            "prior_band", [b_loc, n_heads, L, BANDW], F32, isOutput=True
        )
    else:
        pr_d = nc.declare_dram_parameter(
            "prior", [b_loc, n_heads, L, L], F32, isOutput=True
        )
    vo_d = nc.declare_dram_parameter(
        "vout_t", [b_loc, n_heads, E, L], F32, isOutput=True
    )

    q_ap, k_ap, v_ap, sg_ap = q_d.ap(), k_d.ap(), v_d.ap(), sg_d.ap()
    st_ap, pr_ap, vo_ap = st_d.ap(), pr_d.ap(), vo_d.ap()

    with tile.TileContext(nc) as tc, ExitStack() as ctx:
        consts = ctx.enter_context(tc.tile_pool(name="consts", bufs=1))
        sigp = ctx.enter_context(tc.tile_pool(name="sigp", bufs=1))
        inp = ctx.enter_context(tc.tile_pool(name="inp", bufs=bufs["inp"]))
        qkt = ctx.enter_context(tc.tile_pool(name="qkt", bufs=bufs["qkt"]))
        expp = ctx.enter_context(tc.tile_pool(name="expp", bufs=bufs["expp"]))
        serp = ctx.enter_context(tc.tile_pool(name="serp", bufs=bufs["serp"]))
        prip = ctx.enter_context(tc.tile_pool(name="prip", bufs=bufs["prip"]))
        voutp = ctx.enter_context(tc.tile_pool(name="voutp", bufs=bufs["voutp"]))
        invp = ctx.enter_context(tc.tile_pool(name="invp", bufs=2))
        bcsp = ctx.enter_context(tc.tile_pool(name="bcsp", bufs=2))
        ps_tp = ctx.enter_context(tc.tile_pool(name="ps_tp", bufs=1, space="PSUM"))
        ps_sc = ctx.enter_context(tc.tile_pool(name="ps_sc", bufs=bufs["sc"], space="PSUM"))
        ps_av = ctx.enter_context(tc.tile_pool(name="ps_av", bufs=bufs["av"], space="PSUM"))
        ps_bc = ctx.enter_context(tc.tile_pool(name="ps_bc", bufs=1, space="PSUM"))

        # ---- constants ----
        # dist2[p, lb, j] = (lb*128 + p - j)^2 as f32; single iota (p - j),
        # then DVE shifts/square (gpsimd iota takes few sync waits).
        ii = consts.tile([P, L], I32)
        nc.gpsimd.iota(ii[:, :], pattern=[[-1, L]], base=0, channel_multiplier=1)
        iof = consts.tile([P, L], F32)
        nc.vector.tensor_copy(iof[:, :], ii[:, :])
        dist2 = consts.tile([P, NLB, L], F32)
        for lb in range(NLB):
            nc.vector.tensor_scalar_add(dist2[:, lb, :], iof[:, :], float(lb * P))
        nc.vector.tensor_mul(dist2[:, :, :], dist2[:, :, :], dist2[:, :, :])

        ident = consts.tile([P, P], F32)
        make_identity(nc, ident)

        ones_f = consts.tile([P, P], F32)  # f32 ones source (memset can't write f32r)
        nc.vector.memset(ones_f[:, :], 1.0)
        ones_t = consts.tile([P, P], F32)  # row 64 holds the ones for bcast lhsT
        nc.vector.tensor_copy(r(ones_t[64:65, :]), ones_f[64:65, :])

        # ---- sigma preprocessing for all local batches ----
        # sig[p, lb, b, h];  a = -1/(2*s^2), lnc = -(ln s + ln sqrt(2pi))
        # where s = 3^(sigmoid(5x)+1e-5) - 1
        s3 = sigp.tile([P, NLB, b_loc, H], F32)
        lnc_t = sigp.tile([P, NLB, b_loc, H], F32)
        a_t = sigp.tile([P, NLB, b_loc, H], F32)
        for b in range(b_loc):
            sraw = sigp.tile([P, NLB, H], F32, tag=f"sraw{b}")
            ld_eng.dma_start(
                sraw, sg_ap[b].rearrange("(k p) h -> p k h", p=P)
            )
            s3b = s3[:, :, b, :]
            lncb = lnc_t[:, :, b, :]
            ab = a_t[:, :, b, :]
            nc.scalar.activation(s3b, sraw, AF.Sigmoid, scale=5.0)
            nc.vector.tensor_scalar(
                s3b, s3b, 1e-5, LN3,
                op0=mybir.AluOpType.add, op1=mybir.AluOpType.mult,
            )
            # s = expm1(z): exp path for z >= 0.1; cubic Taylor for small z
            # (avoids the exp(z)-1 cancellation that costs ~1% sigma error
            # at z ~ 1e-5)
            if taylor:
                zt = sigp.tile([P, NLB, H], F32, tag=f"zt{b}")
                nc.vector.tensor_copy(zt[:, :, :], s3b)
            nc.scalar.activation(s3b, s3b, AF.Exp)
            nc.vector.tensor_scalar_add(s3b, s3b, -1.0)
            if taylor:
                _taylor_fix(nc, sigp, s3b, zt, b)

            nc.scalar.activation(lncb, s3b, AF.Ln)
            nc.vector.tensor_scalar(
                lncb, lncb, LN_SQRT_2PI, -1.0,
                op0=mybir.AluOpType.add, op1=mybir.AluOpType.mult,
            )
            nc.vector.tensor_mul(ab, s3b, s3b)
            nc.vector.reciprocal(ab, ab)
            nc.vector.tensor_scalar_mul(ab, ab, -0.5)

        pri_bufs = None
        if band is not None:
            pri_bufs = []
            for i in range(2):
                prib = prip.tile([P, NLB, L], F32, tag=f"prib{i}")
                pri_bufs.append(prib)
            for t in pri_bufs:
                nc.vector.memset(t, 0.0)

        # ---- main loop ----
        for b in range(b_loc):
            # load Q/K naturally: [p(l within blk), lblk, he]
            qnat = inp.tile([P, NLB, L], F32, tag="qnat")
            ld_eng.dma_start(
                qnat, q_ap[b].rearrange("(k p) h e -> p k (h e)", p=P)
            )
            knat = inp.tile([P, NLB, L], F32, tag="knat")
            ld_eng.dma_start(
                knat, k_ap[b].rearrange("(k p) h e -> p k (h e)", p=P)
            )
            # V with a ones column: [p(s within blk), sblk, h, 65]
            vnat = inp.tile([P, NLB, L], F32, tag="vnat")
            ld_eng.dma_start(
                vnat, v_ap[b].rearrange("(k p) h e -> p k (h e)", p=P)
            )
            vaug = inp.tile([P, NLB, H, E + 1], F32, tag="vaug")
            nc.vector.tensor_copy(
                r(vaug[:, :, :, 0:E]),
                vnat[:, :, :].rearrange("p k (h e) -> p k h e", e=E),
            )
            nc.vector.tensor_copy(
                r(vaug[:, :, :, E]),
                ones_f[:, 0 : NLB * H].rearrange("p (k h) -> p k h", k=NLB),
            )

            # PE-transpose Q,K:  qt[j][he_p, l] with he rows j*128..j*128+127
            qts, kts = [], []
            for name, nat, lst in (("q", qnat, qts), ("k", knat, kts)):
                for j in range(NLB):
                    pt = ps_tp.tile([P, L], F32, tag="tp")
                    for i in range(NLB):
                        nc.tensor.transpose(
                            pt[:, i * P : (i + 1) * P],
                            nat[:, i, j * P : (j + 1) * P],
                            ident[:, :],
                        )
                    t = qkt.tile([P, L], F32, tag=f"{name}t{j}")
                    nc.any.tensor_copy(r(t[:, :]), pt[:, :])
                    lst.append(t)

            vout = voutp.tile([E, n_heads, L], F32)

            for h in range(n_heads):
                g, po = h // 2, (h % 2) * 64
                qt_s = qts[g][po : po + 64, :]  # [64, 512] = Q^T for head h
                av = ps_av.tile([E + 1, L], F32)
                exps = []
                for sp in range(2):
                    sc = ps_sc.tile([P, 2, L], F32)
                    for si in range(2):
                        sblk = 2 * sp + si
                        kt_s = kts[g][po : po + 64, sblk * P : (sblk + 1) * P]
                        nc.tensor.matmul(
                            sc[:, si, :], r(kt_s), r(qt_s), start=True, stop=True
                        )
                    ex = expp.tile([P, 2, L], F32, tag="expt")
                    nc.scalar.activation(r(ex[:, :, :]), sc, AF.Exp, scale=SCALE)
                    exps.append(ex)
                    for si in range(2):
                        sblk = 2 * sp + si
                        nc.tensor.matmul(
                            av[:, :],
                            r(vaug[:, sblk, h, :]),
                            r(ex[:, si, :]),
                            start=(sblk == 0),
                            stop=(sblk == NLB - 1),
                        )
                # denominator -> reciprocal -> broadcast to 128 partitions
                inv = invp.tile([65, L], F32)
                with nc.allow_low_precision(reason="fp32r rounding for PE bcast"):
                    nc.vector.reciprocal(r(inv[64:65, :]), av[64:65, :])
                bc = ps_bc.tile([P, L], F32)
                nc.tensor.matmul(
                    bc[:, :], r(ones_t[64:65, :]), r(inv[64:65, :]),
                    start=True, stop=True,
                )
                bcs = bcsp.tile([P, L], F32)
                nc.any.tensor_copy(bcs[:, :], bc[:, :])
                # series (transposed layout [s, l]) + V
                ser = serp.tile([P, NLB, L], F32)
                for sp in range(2):
                    for si in range(2):
                        sblk = 2 * sp + si
                        nc.vector.tensor_mul(
                            ser[:, sblk, :], exps[sp][:, si, :], bcs[:, :]
                        )
                ser_eng.dma_start(
                    st_ap[b, h].rearrange("(k p) l -> p k l", p=P), ser
                )
                nc.vector.tensor_mul(vout[:, h, :], av[0:E, :], bcs[0:E, :])

                # prior
                if band_pack:
                    pri = prip.tile([P, NLB, BANDW], F32)
                    for lb in range(NLB):
                        lo = BAND_LOS[lb]
                        nc.scalar.activation(
                            pri[:, lb, :],
                            dist2[:, lb, lo : lo + BANDW],
                            AF.Exp,
                            scale=a_t[:, lb, b, h : h + 1],
                            bias=lnc_t[:, lb, b, h : h + 1],
                        )
                    pri_eng.dma_start(
                        pr_ap[b, h].rearrange("(k p) w -> p k w", p=P), pri
                    )
                else:
                    if band is None:
                        pri = prip.tile([P, NLB, L], F32)
                    else:
                        pri = pri_bufs[(b * n_heads + h) % 2]
                    for lb in range(NLB):
                        if band is None:
                            lo, hi = 0, L
                        else:
                            lo = max(0, lb * P - band)
                            hi = min(L, (lb + 1) * P + band)
                        nc.scalar.activation(
                            pri[:, lb, lo:hi],
                            dist2[:, lb, lo:hi],
                            AF.Exp,
                            scale=a_t[:, lb, b, h : h + 1],
                            bias=lnc_t[:, lb, b, h : h + 1],
                        )
                    pri_eng.dma_start(
                        pr_ap[b, h].rearrange("(k p) s -> p k s", p=P), pri
                    )

            vo_eng.dma_start(vo_ap[b].rearrange("h d l -> d h l"), vout)

    nc.compile()
    return nc


TRACE = False  # set True (e.g. from test.py) to capture an NTFF profile
LAST_RESULTS = None  # full BassKernelResults of the most recent run
CONFIG = dict(band_pack=True, bufs=dict(serp=4, prip=4, voutp=1), taylor=False,
              out_eng=("sync", "scalar", "sync"))


def kernel(queries, keys, values, sigma):
    global LAST_RESULTS
    from concourse.bass_utils import run_bass_kernel_spmd

    queries = np.ascontiguousarray(queries, dtype=np.float32)
    keys = np.ascontiguousarray(keys, dtype=np.float32)
    values = np.ascontiguousarray(values, dtype=np.float32)
    sigma = np.ascontiguousarray(sigma, dtype=np.float32)

    nc = build_kernel(**CONFIG)
    in_maps = []
    for i in range(N_CORES):
        s = slice(i * B_LOC, (i + 1) * B_LOC)
        in_maps.append(
            {
                "queries": queries[s],
                "keys": keys[s],
                "values": values[s],
                "sigma": sigma[s],
            }
        )
    res = run_bass_kernel_spmd(
        nc, in_maps, core_ids=list(range(N_CORES)), trace=TRACE
    )
    LAST_RESULTS = res
    results = res.results

    series_t = np.concatenate([r_["series_t"] for r_ in results], axis=0)
    vout_t = np.concatenate([r_["vout_t"] for r_ in results], axis=0)
    if CONFIG.get("band_pack"):
        pb = np.concatenate([r_["prior_band"] for r_ in results], axis=0)
        prior = np.zeros((B, H, L, L), dtype=np.float32)
        for lb, lo in enumerate(BAND_LOS):
            prior[:, :, lb * P : (lb + 1) * P, lo : lo + BANDW] = pb[
                :, :, lb * P : (lb + 1) * P, :
            ]
    else:
        prior = np.concatenate([r_["prior"] for r_ in results], axis=0)

    V = vout_t.transpose(0, 3, 1, 2)  # [B,H,E,L] -> [B,L,H,E]
    series = series_t.transpose(0, 1, 3, 2)  # [B,H,S,L] -> [B,H,L,S]
    return V, series, prior
